# revision 25
# baseline (speedup 1.0000x reference)
"""Trainium2 Bass kernel: Bahdanau (additive/tanh) attention + softmax.

Reference computation (per (b, n) pair, B=64, N1=8, K=128, E=256, H=512):
    q_proj = q @ wq^T + wq_b                  # [H]
    k_proj = keys @ wk^T + wk_b               # [K, H]
    hidden = tanh(q_proj + k_proj)            # [K, H]
    scores = hidden @ wv^T + wv_b             # [K, 1]
    scores += mask * -1e9
    attn_w = softmax(scores, axis=K)
    attn_out = sum_k attn_w * keys            # [E]

Sharding: data-parallel over batch. Each of the 8 cores gets 8 batches
= 64 (b, n) pairs; all weights replicated. kernel() shards/pre-transposes
the inputs on the host, runs one SPMD NEFF on cores 0-7 via
run_bass_kernel_spmd, and concatenates the per-core outputs.

Per-core dataflow ("H on partitions", all matmuls in float32r = full PE rate):
  * keysT [e, (bn k)] streams as the matmul moving operand in 1 MB blocks;
    wkT chunks are stationary -> k_proj^T [h_chunk, (bn k)] in PSUM.
  * q_proj^T[h, bn] (+ wk_b + wq_b) is broadcast-added into the PSUM tile by
    the DVE (stride-0 access pattern along k), then ScalarE tanh evacuates
    PSUM -> SBUF ([128, 1024] tiles).
  * scores[1, (bn k)] = wv_chunk (stationary) @ hidden, accumulated over the
    4 h-chunks in PSUM; wv_b is dropped (softmax-invariant).
  * per 16-bn tail group: scores reshaped to [16, 128] (small SBUF->SBUF
    DMA), masked, softmaxed (ScalarE exp with accum_out giving denominators),
    transposed on the PE, scattered to a block-diagonal stationary with one
    gpsimd affine_select, and contracted against keys [k, e] in 16
    accumulating matmuls -> attn_out rows. Tail work is interleaved into the
    next groups' slots via a deferred-emission scheduler so the in-order
    engine queues never stall.

The kernel streams ~12.9 MB/core from HBM, runs ~2.2 GFLOP/core of f32r
matmul, and measures ~108 us/core on hardware (loop-slope timing).
"""

import os
import sys
from contextlib import ExitStack

import numpy as np

if "/opt/trn_rl_repo" not in sys.path:
    sys.path.insert(0, "/opt/trn_rl_repo")

B, N1, K, E, H = 64, 8, 128, 256, 512
NCORES = 8
BS = B // NCORES            # batches per core
NB = BS * N1                # (b, n) pairs per core = 64
EC, ES = 2, 128             # E split into chunks of 128 (contraction tiles)
HC, HS = 4, 128             # H split into chunks of 128 (PSUM partition tiles)
BLK = 512                   # moving-operand columns per block (= 4 bn)
BPB = BLK // K              # bn per block
NBLK = NB * K // BLK        # 16 blocks

_CACHE = {}


def _build_nc(repeat=1, qmode=None):
    # q-add placement: "dve" (DVE broadcast adds), "pe" (row-packed matmuls
    # against the block indicator), or "mix" (DVE for h 0-1, PE for h 2-3 --
    # balances the two near-critical engines)
    if qmode is None:
        qmode = os.environ.get("KERNEL_QMODE", "dve")
    pe_h = {"dve": (), "pe": (0, 1, 2, 3), "mix": (2, 3)}[qmode]
    qdve = len(pe_h) < 4
    import concourse.tile as tile
    from concourse import bacc, mybir

    f32 = mybir.dt.float32
    f32r = mybir.dt.float32r
    i32 = mybir.dt.int32
    AF = mybir.ActivationFunctionType
    AX = mybir.AxisListType
    ALU = mybir.AluOpType

    nc = bacc.Bacc("TRN2", target_bir_lowering=False, debug=False)

    NTG = 4                  # tail groups of TB=16 bn
    TB = NB // NTG
    NGK = NBLK // 2          # ktile groups (block pairs)

    keysT_d = nc.dram_tensor("keysT", [EC, ES, NB * K], f32r, kind="ExternalInput")
    keysN_d = nc.dram_tensor("keysN", [K, NB, E], f32r, kind="ExternalInput")
    qT_d = nc.dram_tensor("qT", [EC, ES, NB], f32r, kind="ExternalInput")
    maskI_d = nc.dram_tensor("maskI", [NB, K], i32, kind="ExternalInput")
    wkT_d = nc.dram_tensor("wkT", [EC, ES, H], f32r, kind="ExternalInput")
    wqT_d = nc.dram_tensor("wqT", [EC, ES, H], f32r, kind="ExternalInput")
    wvT_d = nc.dram_tensor("wvT", [HS, HC], f32r, kind="ExternalInput")
    wkb_d = nc.dram_tensor("wkb", [1, H], f32, kind="ExternalInput")
    wqb_d = nc.dram_tensor("wqb", [1, H], f32, kind="ExternalInput")
    eye_d = nc.dram_tensor("eye16", [TB, TB], f32, kind="ExternalInput")
    ind_d = (nc.dram_tensor("ind2", [2 * NB, NB * K], f32r,
                            kind="ExternalInput") if pe_h else None)

    out_d = nc.dram_tensor("attn_out", [NB, E], f32, kind="ExternalOutput")
    attw_d = nc.dram_tensor("attn_w", [NB, K], f32, kind="ExternalOutput")

    def r(ap):
        return ap if ap.dtype == f32r else ap.bitcast(f32r)

    with tile.TileContext(nc) as tc, ExitStack() as ctx:
        if repeat > 1:
            # timing mode: run the whole body `repeat` times in a hardware
            # loop so device time dominates the host/axon dispatch overhead
            ctx.enter_context(
                tc.For_i(0, repeat, 1, hint_engines=(mybir.EngineType.PE,))
            )
        const = ctx.enter_context(tc.tile_pool(name="const", bufs=1))
        ktp = ctx.enter_context(tc.tile_pool(name="ktp", bufs=3))
        hidp = ctx.enter_context(tc.tile_pool(name="hidp", bufs=6))
        tailp = ctx.enter_context(tc.tile_pool(name="tailp", bufs=2))
        pk = ctx.enter_context(tc.tile_pool(name="pk", bufs=2, space="PSUM"))
        pscore = ctx.enter_context(tc.tile_pool(name="pscore", bufs=1, space="PSUM"))
        pao = ctx.enter_context(tc.tile_pool(name="pao", bufs=1, space="PSUM"))
        pmisc = ctx.enter_context(tc.tile_pool(name="pmisc", bufs=1, space="PSUM"))

        # ---------------- constants / whole-kernel inputs ----------------
        qT_sb = const.tile([ES, EC, NB], f32r)
        nc.sync.dma_start(qT_sb, qT_d.ap().rearrange("c p n -> p c n"))
        wqT_sb = const.tile([ES, EC, H], f32r)
        for c in range(EC):
            nc.sync.dma_start(wqT_sb[:, c, :], wqT_d.ap()[c])
        wkb_sb = const.tile([1, H], f32)
        nc.sync.dma_start(wkb_sb, wkb_d.ap())
        wqb_sb = const.tile([1, H], f32)
        nc.sync.dma_start(wqb_sb, wqb_d.ap())
        wkT_sb = const.tile([ES, EC, H], f32r)
        nc.sync.dma_start(wkT_sb, wkT_d.ap().rearrange("c p h -> p c h"))
        wvT_sb = const.tile([HS, HC], f32r)
        nc.sync.dma_start(wvT_sb, wvT_d.ap())
        # ind2: block-indicator stacked twice (rows 0-63 and 64-127) for the
        # row-packed pair of q-add matmuls; row 0 starts with K ones.
        # Loaded in column chunks (SWDGE, parallel to the HWDGE stream) so the
        # first q-add matmul only depends on the first chunk.
        IC = NB * K // 4
        if pe_h:
            ind_sb = const.tile([2 * NB, NB * K], f32r)
            for c in range(2):
                nc.gpsimd.dma_start(out=ind_sb[:, c * IC:(c + 1) * IC],
                                    in_=ind_d.ap()[:, c * IC:(c + 1) * IC])
        eye_sb = const.tile([TB, TB], f32)
        nc.sync.dma_start(eye_sb, eye_d.ap())
        maskf = []
        for t in range(NTG):
            mt = const.tile([TB, K], f32, tag=f"maskf{t}", name=f"maskf{t}")
            nc.gpsimd.dma_start(out=mt, in_=maskI_d.ap()[t * TB:(t + 1) * TB, :])
            maskf.append(mt)

        keysN_sb = const.tile([K, NB, E], f32r)

        if pe_h:
            ones1 = ind_sb[0:1, 0:NB]
            bsum = const.tile([1, H], f32r)
            nc.vector.tensor_add(bsum, wkb_sb, wqb_sb)

        # ---------------- q projection ----------------
        if True:
            # qpT[h, bn] (+ biases) for DVE broadcast-adds into PSUM
            bsumT = const.tile([HS, HC], f32)
            nc.sync.dma_start(
                bsumT, wkb_d.ap().rearrange("o (c p) -> p (o c)", p=HS))
            wqbT = const.tile([HS, HC], f32)
            nc.sync.dma_start(
                wqbT, wqb_d.ap().rearrange("o (c p) -> p (o c)", p=HS))
            nc.vector.tensor_add(bsumT, bsumT, wqbT)
            qpT_ps = pmisc.tile([HS, HC, NB], f32, tag="pm")
            for hh in range(HC):
                for e in range(EC):
                    nc.tensor.matmul(
                        qpT_ps[:, hh, :],
                        r(wqT_sb[:, e, hh * HS:(hh + 1) * HS]),
                        r(qT_sb[:, e, :]),
                        start=(e == 0), stop=(e == EC - 1))
            qpT_sb = const.tile([HS, HC, NB], f32)
            for hh in range(HC):
                nc.vector.tensor_scalar_add(qpT_sb[:, hh, :], qpT_ps[:, hh, :],
                                            bsumT[:, hh:hh + 1])
        if pe_h:
            qp_ps = pmisc.tile([NB, H], f32, tag="pm")
            nc.tensor.matmul(qp_ps, r(qT_sb[:, 0, :]), r(wqT_sb[:, 0, :]),
                             start=True, stop=False)
            nc.tensor.matmul(qp_ps, r(qT_sb[:, 1, :]), r(wqT_sb[:, 1, :]),
                             start=False, stop=False)
            nc.tensor.matmul(qp_ps, r(ones1), r(bsum), start=False, stop=True)
            qp2_sb = const.tile([2 * NB, H], f32r)
            nc.vector.tensor_copy(qp2_sb[0:NB, :], qp_ps)
            nc.vector.tensor_copy(qp2_sb[NB:2 * NB, :], qp_ps)

        scores_row = const.tile([1, NBLK * BLK], f32)

        # Deferred-emission scheduler: closures registered at (group, h) flush
        # points so the in-order PE queue never waits on ACT/DVE results of
        # the tile it just produced, and the softmax/attn_out tail of each
        # 16-bn group is spread across the next groups' slots.
        sched = {}

        def at(gk, h, fn):
            sched.setdefault((gk, h), []).append(fn)

        def flush(gk, h):
            for fn in sched.pop((gk, h), []):
                fn()

        state = {}

        def make_sc_mm(gk, half, h, hd):
            def f():
                key = f"sc{gk}"
                if key not in state:
                    state[key] = pscore.tile([1, 2 * BLK], f32, tag="sc",
                                             name=f"sc_ps_{gk}")
                sc = state[key][:, half * BLK:(half + 1) * BLK]
                nc.tensor.matmul(sc, r(wvT_sb[:, h:h + 1]), r(hd),
                                 start=(h == 0), stop=(h == HC - 1),
                                 skip_group_check=True)
            return f

        def make_sc_evac(gk):
            def f():
                sc = state.pop(f"sc{gk}")
                nc.scalar.copy(
                    scores_row[:, 2 * gk * BLK:(2 * gk + 2) * BLK], sc)
            return f

        def make_tail_a(t):
            def f():
                sb = tailp.tile([TB, K], f32, tag="scbk", name=f"scbk{t}")
                nc.sync.dma_start(
                    sb,
                    scores_row[0:1, t * TB * K:(t + 1) * TB * K].rearrange(
                        "p (n k) -> p n k", k=K))
                ms = tailp.tile([TB, K], f32, tag="maskS", name=f"maskS{t}")
                nc.vector.tensor_scalar_mul(ms, maskf[t], -1.0e9)
                nc.vector.tensor_add(sb, sb, ms)
                nm = tailp.tile([TB, 1], f32, tag="negmx", name=f"negmx{t}")
                nc.vector.tensor_reduce(nm, sb, axis=AX.X, op=ALU.max,
                                        negate=True)
                state[f"sb{t}"] = sb
                state[f"nm{t}"] = nm
            return f

        def make_tail_b(t):
            def f():
                sb = state.pop(f"sb{t}")
                nm = state.pop(f"nm{t}")
                aw = tailp.tile([TB, K], f32, tag="aw", name=f"aw{t}")
                se = tailp.tile([TB, 1], f32, tag="se", name=f"se{t}")
                nc.scalar.activation(aw, sb, AF.Exp, bias=nm, accum_out=se)
                rs = tailp.tile([TB, 1], f32, tag="rs", name=f"rs{t}")
                nc.vector.reciprocal(rs, se)
                nc.vector.tensor_scalar_mul(aw, aw, rs)
                nc.sync.dma_start(attw_d.ap()[t * TB:(t + 1) * TB, :], aw)
                state[f"aw{t}"] = aw
            return f

        def make_tail_c(t):
            def f():
                aw = state.pop(f"aw{t}")
                tp = pmisc.tile([K, TB], f32, tag="pm", name=f"tp{t}")
                nc.tensor.transpose(tp, aw, eye_sb)
                at_sb = tailp.tile([K, TB], f32r, tag="at", name=f"at{t}")
                nc.vector.tensor_copy(at_sb, tp)
                ad = tailp.tile([K, TB, TB], f32r, tag="ad", name=f"ad{t}")
                nc.gpsimd.affine_select(
                    out=ad, in_=at_sb.unsqueeze(2).broadcast_to([K, TB, TB]),
                    pattern=[[1, TB], [-1, TB]], base=0, channel_multiplier=0,
                    compare_op=ALU.is_equal, fill=0.0)
                state[f"ad{t}"] = ad
            return f

        def make_tail_d(t):
            def f():
                ad = state.pop(f"ad{t}")
                ao = pao.tile([TB, E], f32, tag="ao", name=f"ao{t}")
                for i in range(TB):
                    nc.tensor.matmul(ao, r(ad[:, i, :]),
                                     r(keysN_sb[:, t * TB + i, :]),
                                     start=(i == 0), stop=(i == TB - 1))
                osb = tailp.tile([TB, E], f32, tag="osb", name=f"osb{t}")
                nc.vector.tensor_copy(osb, ao)
                nc.sync.dma_start(out_d.ap()[t * TB:(t + 1) * TB, :], osb)
            return f

        # ---------------- main loop over ktile-group (pair of blocks) ------
        for gk in range(NGK):
            j0, j1 = 2 * gk, 2 * gk + 1
            ktile = ktp.tile([ES, EC, 2 * BLK], f32r, tag="ktile")
            ksl = keysT_d.ap()[:, :, j0 * BLK:(j1 + 1) * BLK]
            if gk == 0:
                for c in range(EC):
                    nc.sync.dma_start(ktile[:, c, :], ksl[c])
            else:
                nc.sync.dma_start(ktile, ksl.rearrange("c p n -> p c n"))
            if gk in (2, 4) and pe_h:
                c = gk // 2 + 1
                nc.gpsimd.dma_start(out=ind_sb[:, c * IC:(c + 1) * IC],
                                    in_=ind_d.ap()[:, c * IC:(c + 1) * IC])
            if gk % 2 == 0:
                t = gk // 2
                nc.gpsimd.dma_start(
                    out=keysN_sb[:, t * TB:(t + 1) * TB, :],
                    in_=keysN_d.ap()[:, t * TB:(t + 1) * TB, :])
            for h in range(HC):
                hsl = slice(h * HS, (h + 1) * HS)
                kp = pk.tile([ES, 2 * BLK], f32, tag="kp", name=f"kp_{gk}_{h}")
                kp0, kp1 = kp[:, 0:BLK], kp[:, BLK:2 * BLK]
                nc.tensor.matmul(kp0, r(wkT_sb[:, 0, hsl]),
                                 r(ktile[:, 0, 0:BLK]), start=True, stop=False)
                nc.tensor.matmul(kp1, r(wkT_sb[:, 0, hsl]),
                                 r(ktile[:, 0, BLK:2 * BLK]),
                                 start=True, stop=False)
                dve_h = h not in pe_h
                nc.tensor.matmul(kp0, r(wkT_sb[:, 1, hsl]),
                                 r(ktile[:, 1, 0:BLK]), start=False, stop=dve_h)
                nc.tensor.matmul(kp1, r(wkT_sb[:, 1, hsl]),
                                 r(ktile[:, 1, BLK:2 * BLK]),
                                 start=False, stop=dve_h)
                if dve_h:
                    qb = qpT_sb[:, h, 8 * gk:8 * gk + 8].unsqueeze(
                        2).broadcast_to([HS, 8, K])
                    nc.vector.tensor_add(
                        kp.rearrange("p (n k) -> p n k", k=K), kp.rearrange(
                            "p (n k) -> p n k", k=K), qb)
                else:
                    # row-packed pair: partitions 0-63/64-127 run concurrently
                    nc.tensor.matmul(kp0, r(qp2_sb[0:NB, hsl]),
                                     r(ind_sb[0:NB, j0 * BLK:(j0 + 1) * BLK]),
                                     start=False, stop=True)
                    nc.tensor.matmul(kp1, r(qp2_sb[NB:2 * NB, hsl]),
                                     r(ind_sb[NB:2 * NB, j1 * BLK:(j1 + 1) * BLK]),
                                     start=False, stop=True)
                hid = hidp.tile([ES, 2 * BLK], f32r, tag="hid")
                nc.scalar.activation(hid, kp, AF.Tanh)
                flush(gk, h)
                nxt = (gk, h + 1) if h + 1 < HC else (gk + 1, 0)
                at(*nxt, make_sc_mm(gk, 0, h, hid[:, 0:BLK]))
                at(*nxt, make_sc_mm(gk, 1, h, hid[:, BLK:2 * BLK]))
            at(gk + 1, 0, make_sc_evac(gk))
            if gk % 2 == 1:
                t = gk // 2
                at(gk + 1, 1, make_tail_a(t))
                at(gk + 1, 2, make_tail_b(t))
                at(gk + 1, 3, make_tail_c(t))
                at(gk + 2, 0, make_tail_d(t))

        # post-loop: flush everything remaining in registration order
        for key in sorted(sched.keys()):
            for fn in list(sched.pop(key)):
                fn()

    nc.compile()
    return nc


def _make_in_maps(inputs):
    queries = np.asarray(inputs["queries"], np.float32)
    keys = np.asarray(inputs["keys"], np.float32)
    masks = np.asarray(inputs["cum_subs_masks"], np.int32)
    wk_w = np.asarray(inputs["wk_w"], np.float32)
    wk_b = np.asarray(inputs["wk_b"], np.float32)
    wq_w = np.asarray(inputs["wq_w"], np.float32)
    wq_b = np.asarray(inputs["wq_b"], np.float32)
    wv_w = np.asarray(inputs["wv_w"], np.float32)
    wv_b = np.asarray(inputs["wv_b"], np.float32)

    wkT = np.ascontiguousarray(wk_w.T).reshape(EC, ES, H)
    wqT = np.ascontiguousarray(wq_w.T).reshape(EC, ES, H)
    wvT = np.ascontiguousarray(wv_w.reshape(HC, HS).T)
    wkb = np.ascontiguousarray(wk_b.reshape(1, H))
    wqb = np.ascontiguousarray(wq_b.reshape(1, H))
    wvb = np.ascontiguousarray(wv_b.reshape(1, 1))
    eye = np.eye(16, dtype=np.float32)
    ind = np.kron(np.eye(NB, dtype=np.float32), np.ones((1, K), np.float32))
    ind2 = np.ascontiguousarray(np.vstack([ind, ind]))

    in_maps = []
    for c in range(NCORES):
        sl = slice(c * BS, (c + 1) * BS)
        kc = keys[sl].reshape(NB, K, E)
        keysT = np.ascontiguousarray(kc.transpose(2, 0, 1)).reshape(EC, ES, NB * K)
        keysN = np.ascontiguousarray(kc.transpose(1, 0, 2))
        qT = np.ascontiguousarray(queries[sl].reshape(NB, E).T).reshape(EC, ES, NB)
        maskI = np.ascontiguousarray(masks[sl].reshape(NB, K))
        in_maps.append(
            {
                "keysT": keysT,
                "keysN": keysN,
                "qT": qT,
                "maskI": maskI,
                "wkT": wkT,
                "wqT": wqT,
                "wvT": wvT,
                "wkb": wkb,
                "wqb": wqb,
                "wvb": wvb,
                "eye16": eye,
                "ind2": ind2,
            }
        )
    return in_maps


def _declared_inputs(nc):
    from concourse import mybir

    names = set()
    for alloc in nc.m.functions[0].allocations:
        if isinstance(alloc, mybir.MemoryLocationSet) and alloc.kind == "ExternalInput":
            names.add(alloc.memorylocations[0].name)
    return names


def _gather(results):
    attn_out = np.concatenate(
        [np.asarray(r["attn_out"]).reshape(BS, N1, E) for r in results], axis=0
    )
    attn_w = np.concatenate(
        [np.asarray(r["attn_w"]).reshape(BS, N1, K, 1) for r in results], axis=0
    )
    return attn_out, attn_w


def kernel(**inputs):
    from concourse.bass_utils import run_bass_kernel_spmd

    nc = _CACHE.get("nc")
    if nc is None:
        nc = _build_nc()
        _CACHE["nc"] = nc

    in_maps = _make_in_maps(inputs)
    declared = _declared_inputs(nc)
    in_maps = [{k: v for k, v in m.items() if k in declared} for m in in_maps]
    trace = bool(int(os.environ.get("KERNEL_TRACE", "0")))
    try:
        res = run_bass_kernel_spmd(
            nc, in_maps, core_ids=list(range(NCORES)), trace=trace
        )
    except ModuleNotFoundError:
        # NTFF profile hook unavailable in this container; run without trace.
        res = run_bass_kernel_spmd(
            nc, in_maps, core_ids=list(range(NCORES)), trace=False
        )
    _CACHE["last_result"] = res
    return _gather(res.results)


# revision 26
# speedup vs baseline: 1.1401x; 1.1401x over previous
"""Trainium2 Bass kernel: Bahdanau (additive/tanh) attention + softmax.

Reference computation (per (b, n) pair, B=64, N1=8, K=128, E=256, H=512):
    q_proj = q @ wq^T + wq_b                  # [H]
    k_proj = keys @ wk^T + wk_b               # [K, H]
    hidden = tanh(q_proj + k_proj)            # [K, H]
    scores = hidden @ wv^T + wv_b             # [K, 1]
    scores += mask * -1e9
    attn_w = softmax(scores, axis=K)
    attn_out = sum_k attn_w * keys            # [E]

Sharding: data-parallel over batch. Each of the 8 cores gets 8 batches
= 64 (b, n) pairs; all weights replicated. kernel() shards/pre-transposes
the inputs on the host, runs one SPMD NEFF on cores 0-7 via
run_bass_kernel_spmd, and concatenates the per-core outputs.

Per-core dataflow ("H on partitions", all matmuls in float32r = full PE rate):
  * keysT [e, (bn k)] streams as the matmul moving operand in 1 MB blocks;
    wkT chunks are stationary -> k_proj^T [h_chunk, (bn k)] in PSUM.
  * q_proj^T[h, bn] (+ wk_b + wq_b) is broadcast-added into the PSUM tile by
    the DVE (stride-0 access pattern along k), then ScalarE tanh evacuates
    PSUM -> SBUF ([128, 1024] tiles).
  * scores[1, (bn k)] = wv_chunk (stationary) @ hidden, accumulated over the
    4 h-chunks in PSUM; wv_b is dropped (softmax-invariant).
  * per 16-bn tail group: scores reshaped to [16, 128] (small SBUF->SBUF
    DMA), masked, softmaxed (ScalarE exp with accum_out giving denominators),
    transposed on the PE, scattered to a block-diagonal stationary with one
    gpsimd affine_select, and contracted against keys [k, e] in 16
    accumulating matmuls -> attn_out rows. Tail work is interleaved into the
    next groups' slots via a deferred-emission scheduler so the in-order
    engine queues never stall.

The kernel streams ~12.9 MB/core from HBM, runs ~2.2 GFLOP/core of f32r
matmul, and measures ~108 us/core on hardware (loop-slope timing).
"""

import os
import sys
from contextlib import ExitStack

import numpy as np

if "/opt/trn_rl_repo" not in sys.path:
    sys.path.insert(0, "/opt/trn_rl_repo")

B, N1, K, E, H = 64, 8, 128, 256, 512
NCORES = 8
BS = B // NCORES            # batches per core
NB = BS * N1                # (b, n) pairs per core = 64
EC, ES = 2, 128             # E split into chunks of 128 (contraction tiles)
HC, HS = 4, 128             # H split into chunks of 128 (PSUM partition tiles)
BLK = 512                   # moving-operand columns per block (= 4 bn)
BPB = BLK // K              # bn per block
NBLK = NB * K // BLK        # 16 blocks

_CACHE = {}


def _build_nc(repeat=1, qmode=None):
    # q-add placement: "dve" (DVE broadcast adds), "pe" (row-packed matmuls
    # against the block indicator), or "mix" (DVE for h 0-1, PE for h 2-3 --
    # balances the two near-critical engines)
    if qmode is None:
        qmode = os.environ.get("KERNEL_QMODE", "dve")
    pe_h = {"dve": (), "pe": (0, 1, 2, 3), "mix": (2, 3)}[qmode]
    qdve = len(pe_h) < 4
    import concourse.tile as tile
    from concourse import bacc, mybir

    f32 = mybir.dt.float32
    f32r = mybir.dt.float32r
    i32 = mybir.dt.int32
    AF = mybir.ActivationFunctionType
    AX = mybir.AxisListType
    ALU = mybir.AluOpType

    nc = bacc.Bacc("TRN2", target_bir_lowering=False, debug=False)

    NTG = 4                  # tail groups of TB=16 bn
    TB = NB // NTG
    NGK = NBLK // 2          # ktile groups (block pairs)

    keysT_d = nc.dram_tensor("keysT", [EC, ES, NB * K], f32r, kind="ExternalInput")
    keysN_d = nc.dram_tensor("keysN", [K, NB, E], f32r, kind="ExternalInput")
    qT_d = nc.dram_tensor("qT", [EC, ES, NB], f32r, kind="ExternalInput")
    maskI_d = nc.dram_tensor("maskI", [NB, K], i32, kind="ExternalInput")
    wkT_d = nc.dram_tensor("wkT", [EC, ES, H], f32r, kind="ExternalInput")
    wqT_d = nc.dram_tensor("wqT", [EC, ES, H], f32r, kind="ExternalInput")
    wvT_d = nc.dram_tensor("wvT", [HS, HC], f32r, kind="ExternalInput")
    wkb_d = nc.dram_tensor("wkb", [1, H], f32, kind="ExternalInput")
    wqb_d = nc.dram_tensor("wqb", [1, H], f32, kind="ExternalInput")
    eye_d = nc.dram_tensor("eye16", [TB, TB], f32, kind="ExternalInput")
    ind_d = (nc.dram_tensor("ind2", [2 * NB, NB * K], f32r,
                            kind="ExternalInput") if pe_h else None)

    out_d = nc.dram_tensor("attn_out", [NB, E], f32, kind="ExternalOutput")
    attw_d = nc.dram_tensor("attn_w", [NB, K], f32, kind="ExternalOutput")

    def r(ap):
        return ap if ap.dtype == f32r else ap.bitcast(f32r)

    with tile.TileContext(nc) as tc, ExitStack() as ctx:
        if repeat > 1:
            # timing mode: run the whole body `repeat` times in a hardware
            # loop so device time dominates the host/axon dispatch overhead
            ctx.enter_context(
                tc.For_i(0, repeat, 1, hint_engines=(mybir.EngineType.PE,))
            )
        const = ctx.enter_context(tc.tile_pool(name="const", bufs=1))
        ktp = ctx.enter_context(tc.tile_pool(name="ktp", bufs=3))
        hidp = ctx.enter_context(tc.tile_pool(name="hidp", bufs=6))
        tailp = ctx.enter_context(tc.tile_pool(name="tailp", bufs=2))
        pk = ctx.enter_context(tc.tile_pool(name="pk", bufs=2, space="PSUM"))
        pscore = ctx.enter_context(tc.tile_pool(name="pscore", bufs=1, space="PSUM"))
        pao = ctx.enter_context(tc.tile_pool(name="pao", bufs=1, space="PSUM"))
        pmisc = ctx.enter_context(tc.tile_pool(name="pmisc", bufs=1, space="PSUM"))

        # ---------------- constants / whole-kernel inputs ----------------
        qT_sb = const.tile([ES, EC, NB], f32r)
        nc.sync.dma_start(qT_sb, qT_d.ap().rearrange("c p n -> p c n"))
        wqT_sb = const.tile([ES, EC, H], f32r)
        for c in range(EC):
            nc.sync.dma_start(wqT_sb[:, c, :], wqT_d.ap()[c])
        wkb_sb = const.tile([1, H], f32)
        nc.sync.dma_start(wkb_sb, wkb_d.ap())
        wqb_sb = const.tile([1, H], f32)
        nc.sync.dma_start(wqb_sb, wqb_d.ap())
        wkT_sb = const.tile([ES, EC, H], f32r)
        nc.sync.dma_start(wkT_sb, wkT_d.ap().rearrange("c p h -> p c h"))
        wvT_sb = const.tile([HS, HC], f32r)
        nc.sync.dma_start(wvT_sb, wvT_d.ap())
        # ind2: block-indicator stacked twice (rows 0-63 and 64-127) for the
        # row-packed pair of q-add matmuls; row 0 starts with K ones.
        # Loaded in column chunks (SWDGE, parallel to the HWDGE stream) so the
        # first q-add matmul only depends on the first chunk.
        IC = NB * K // 4
        if pe_h:
            ind_sb = const.tile([2 * NB, NB * K], f32r)
            for c in range(2):
                nc.gpsimd.dma_start(out=ind_sb[:, c * IC:(c + 1) * IC],
                                    in_=ind_d.ap()[:, c * IC:(c + 1) * IC])
        eye_sb = const.tile([TB, TB], f32)
        nc.sync.dma_start(eye_sb, eye_d.ap())
        maskf = []
        for t in range(NTG):
            mt = const.tile([TB, K], f32, tag=f"maskf{t}", name=f"maskf{t}")
            nc.gpsimd.dma_start(out=mt, in_=maskI_d.ap()[t * TB:(t + 1) * TB, :])
            maskf.append(mt)

        keysN_sb = const.tile([K, NB, E], f32r)

        if pe_h:
            ones1 = ind_sb[0:1, 0:NB]
            bsum = const.tile([1, H], f32r)
            nc.vector.tensor_add(bsum, wkb_sb, wqb_sb)

        # ---------------- q projection ----------------
        if True:
            # qpT[h, bn] (+ biases) for DVE broadcast-adds into PSUM
            bsumT = const.tile([HS, HC], f32)
            nc.sync.dma_start(
                bsumT, wkb_d.ap().rearrange("o (c p) -> p (o c)", p=HS))
            wqbT = const.tile([HS, HC], f32)
            nc.sync.dma_start(
                wqbT, wqb_d.ap().rearrange("o (c p) -> p (o c)", p=HS))
            nc.vector.tensor_add(bsumT, bsumT, wqbT)
            qpT_ps = pmisc.tile([HS, HC, NB], f32, tag="pm")
            for hh in range(HC):
                for e in range(EC):
                    nc.tensor.matmul(
                        qpT_ps[:, hh, :],
                        r(wqT_sb[:, e, hh * HS:(hh + 1) * HS]),
                        r(qT_sb[:, e, :]),
                        start=(e == 0), stop=(e == EC - 1))
            qpT_sb = const.tile([HS, HC, NB], f32)
            for hh in range(HC):
                nc.vector.tensor_scalar_add(qpT_sb[:, hh, :], qpT_ps[:, hh, :],
                                            bsumT[:, hh:hh + 1])
        if pe_h:
            qp_ps = pmisc.tile([NB, H], f32, tag="pm")
            nc.tensor.matmul(qp_ps, r(qT_sb[:, 0, :]), r(wqT_sb[:, 0, :]),
                             start=True, stop=False)
            nc.tensor.matmul(qp_ps, r(qT_sb[:, 1, :]), r(wqT_sb[:, 1, :]),
                             start=False, stop=False)
            nc.tensor.matmul(qp_ps, r(ones1), r(bsum), start=False, stop=True)
            qp2_sb = const.tile([2 * NB, H], f32r)
            nc.vector.tensor_copy(qp2_sb[0:NB, :], qp_ps)
            nc.vector.tensor_copy(qp2_sb[NB:2 * NB, :], qp_ps)

        scores_row = const.tile([1, NBLK * BLK], f32)

        # Deferred-emission scheduler: closures registered at (group, h) flush
        # points so the in-order PE queue never waits on ACT/DVE results of
        # the tile it just produced, and the softmax/attn_out tail of each
        # 16-bn group is spread across the next groups' slots.
        sched = {}

        def at(gk, h, fn):
            sched.setdefault((gk, h), []).append(fn)

        def flush(gk, h):
            for fn in sched.pop((gk, h), []):
                fn()

        state = {}

        def make_sc_mm(gk, half, h, hd):
            def f():
                key = f"sc{gk}"
                if key not in state:
                    state[key] = pscore.tile([1, 2 * BLK], f32, tag="sc",
                                             name=f"sc_ps_{gk}")
                sc = state[key][:, half * BLK:(half + 1) * BLK]
                nc.tensor.matmul(sc, r(wvT_sb[:, h:h + 1]), r(hd),
                                 start=(h == 0), stop=(h == HC - 1),
                                 skip_group_check=True)
            return f

        def make_sc_evac(gk):
            def f():
                sc = state.pop(f"sc{gk}")
                nc.scalar.copy(
                    scores_row[:, 2 * gk * BLK:(2 * gk + 2) * BLK], sc)
            return f

        def make_tail_a(t):
            def f():
                sb = tailp.tile([TB, K], f32, tag="scbk", name=f"scbk{t}")
                nc.sync.dma_start(
                    sb,
                    scores_row[0:1, t * TB * K:(t + 1) * TB * K].rearrange(
                        "p (n k) -> p n k", k=K))
                ms = tailp.tile([TB, K], f32, tag="maskS", name=f"maskS{t}")
                nc.vector.tensor_scalar_mul(ms, maskf[t], -1.0e9)
                nc.vector.tensor_add(sb, sb, ms)
                nm = tailp.tile([TB, 1], f32, tag="negmx", name=f"negmx{t}")
                nc.vector.tensor_reduce(nm, sb, axis=AX.X, op=ALU.max,
                                        negate=True)
                state[f"sb{t}"] = sb
                state[f"nm{t}"] = nm
            return f

        def make_tail_b(t):
            def f():
                sb = state.pop(f"sb{t}")
                nm = state.pop(f"nm{t}")
                aw = tailp.tile([TB, K], f32, tag="aw", name=f"aw{t}")
                se = tailp.tile([TB, 1], f32, tag="se", name=f"se{t}")
                nc.scalar.activation(aw, sb, AF.Exp, bias=nm, accum_out=se)
                rs = tailp.tile([TB, 1], f32, tag="rs", name=f"rs{t}")
                nc.vector.reciprocal(rs, se)
                nc.vector.tensor_scalar_mul(aw, aw, rs)
                nc.sync.dma_start(attw_d.ap()[t * TB:(t + 1) * TB, :], aw)
                state[f"aw{t}"] = aw
            return f

        def make_tail_c(t):
            def f():
                aw = state.pop(f"aw{t}")
                tp = pmisc.tile([K, TB], f32, tag="pm", name=f"tp{t}")
                nc.tensor.transpose(tp, aw, eye_sb)
                at_sb = tailp.tile([K, TB], f32r, tag="at", name=f"at{t}")
                nc.vector.tensor_copy(at_sb, tp)
                ad = tailp.tile([K, TB, TB], f32r, tag="ad", name=f"ad{t}")
                nc.gpsimd.affine_select(
                    out=ad, in_=at_sb.unsqueeze(2).broadcast_to([K, TB, TB]),
                    pattern=[[1, TB], [-1, TB]], base=0, channel_multiplier=0,
                    compare_op=ALU.is_equal, fill=0.0)
                state[f"ad{t}"] = ad
            return f

        def make_tail_d(t):
            def f():
                ad = state.pop(f"ad{t}")
                ao = pao.tile([TB, E], f32, tag="ao", name=f"ao{t}")
                for i in range(TB):
                    nc.tensor.matmul(ao, r(ad[:, i, :]),
                                     r(keysN_sb[:, t * TB + i, :]),
                                     start=(i == 0), stop=(i == TB - 1))
                osb = tailp.tile([TB, E], f32, tag="osb", name=f"osb{t}")
                nc.vector.tensor_copy(osb, ao)
                nc.sync.dma_start(out_d.ap()[t * TB:(t + 1) * TB, :], osb)
            return f

        # ---------------- main loop over ktile-group (pair of blocks) ------
        for gk in range(NGK):
            j0, j1 = 2 * gk, 2 * gk + 1
            ktile = ktp.tile([ES, EC, 2 * BLK], f32r, tag="ktile")
            ksl = keysT_d.ap()[:, :, j0 * BLK:(j1 + 1) * BLK]
            if gk == 0:
                for c in range(EC):
                    nc.sync.dma_start(ktile[:, c, :], ksl[c])
            else:
                nc.sync.dma_start(ktile, ksl.rearrange("c p n -> p c n"))
            if gk in (2, 4) and pe_h:
                c = gk // 2 + 1
                nc.gpsimd.dma_start(out=ind_sb[:, c * IC:(c + 1) * IC],
                                    in_=ind_d.ap()[:, c * IC:(c + 1) * IC])
            if gk % 2 == 0:
                t = gk // 2
                nc.sync.dma_start(
                    keysN_sb[:, t * TB:(t + 1) * TB, :],
                    keysN_d.ap()[:, t * TB:(t + 1) * TB, :])
            for h in range(HC):
                hsl = slice(h * HS, (h + 1) * HS)
                kp = pk.tile([ES, 2 * BLK], f32, tag="kp", name=f"kp_{gk}_{h}")
                kp0, kp1 = kp[:, 0:BLK], kp[:, BLK:2 * BLK]
                nc.tensor.matmul(kp0, r(wkT_sb[:, 0, hsl]),
                                 r(ktile[:, 0, 0:BLK]), start=True, stop=False)
                nc.tensor.matmul(kp1, r(wkT_sb[:, 0, hsl]),
                                 r(ktile[:, 0, BLK:2 * BLK]),
                                 start=True, stop=False)
                dve_h = h not in pe_h
                nc.tensor.matmul(kp0, r(wkT_sb[:, 1, hsl]),
                                 r(ktile[:, 1, 0:BLK]), start=False, stop=dve_h)
                nc.tensor.matmul(kp1, r(wkT_sb[:, 1, hsl]),
                                 r(ktile[:, 1, BLK:2 * BLK]),
                                 start=False, stop=dve_h)
                if dve_h:
                    qb = qpT_sb[:, h, 8 * gk:8 * gk + 8].unsqueeze(
                        2).broadcast_to([HS, 8, K])
                    nc.vector.tensor_add(
                        kp.rearrange("p (n k) -> p n k", k=K), kp.rearrange(
                            "p (n k) -> p n k", k=K), qb)
                else:
                    # row-packed pair: partitions 0-63/64-127 run concurrently
                    nc.tensor.matmul(kp0, r(qp2_sb[0:NB, hsl]),
                                     r(ind_sb[0:NB, j0 * BLK:(j0 + 1) * BLK]),
                                     start=False, stop=True)
                    nc.tensor.matmul(kp1, r(qp2_sb[NB:2 * NB, hsl]),
                                     r(ind_sb[NB:2 * NB, j1 * BLK:(j1 + 1) * BLK]),
                                     start=False, stop=True)
                hid = hidp.tile([ES, 2 * BLK], f32r, tag="hid")
                nc.scalar.activation(hid, kp, AF.Tanh)
                flush(gk, h)
                nxt = (gk, h + 1) if h + 1 < HC else (gk + 1, 0)
                at(*nxt, make_sc_mm(gk, 0, h, hid[:, 0:BLK]))
                at(*nxt, make_sc_mm(gk, 1, h, hid[:, BLK:2 * BLK]))
            at(gk + 1, 0, make_sc_evac(gk))
            if gk % 2 == 1:
                t = gk // 2
                at(gk + 1, 1, make_tail_a(t))
                at(gk + 1, 2, make_tail_b(t))
                at(gk + 1, 3, make_tail_c(t))
                at(gk + 2, 0, make_tail_d(t))

        # post-loop: flush everything remaining in registration order
        for key in sorted(sched.keys()):
            for fn in list(sched.pop(key)):
                fn()

    nc.compile()
    return nc


def _make_in_maps(inputs):
    queries = np.asarray(inputs["queries"], np.float32)
    keys = np.asarray(inputs["keys"], np.float32)
    masks = np.asarray(inputs["cum_subs_masks"], np.int32)
    wk_w = np.asarray(inputs["wk_w"], np.float32)
    wk_b = np.asarray(inputs["wk_b"], np.float32)
    wq_w = np.asarray(inputs["wq_w"], np.float32)
    wq_b = np.asarray(inputs["wq_b"], np.float32)
    wv_w = np.asarray(inputs["wv_w"], np.float32)
    wv_b = np.asarray(inputs["wv_b"], np.float32)

    wkT = np.ascontiguousarray(wk_w.T).reshape(EC, ES, H)
    wqT = np.ascontiguousarray(wq_w.T).reshape(EC, ES, H)
    wvT = np.ascontiguousarray(wv_w.reshape(HC, HS).T)
    wkb = np.ascontiguousarray(wk_b.reshape(1, H))
    wqb = np.ascontiguousarray(wq_b.reshape(1, H))
    wvb = np.ascontiguousarray(wv_b.reshape(1, 1))
    eye = np.eye(16, dtype=np.float32)
    ind = np.kron(np.eye(NB, dtype=np.float32), np.ones((1, K), np.float32))
    ind2 = np.ascontiguousarray(np.vstack([ind, ind]))

    in_maps = []
    for c in range(NCORES):
        sl = slice(c * BS, (c + 1) * BS)
        kc = keys[sl].reshape(NB, K, E)
        keysT = np.ascontiguousarray(kc.transpose(2, 0, 1)).reshape(EC, ES, NB * K)
        keysN = np.ascontiguousarray(kc.transpose(1, 0, 2))
        qT = np.ascontiguousarray(queries[sl].reshape(NB, E).T).reshape(EC, ES, NB)
        maskI = np.ascontiguousarray(masks[sl].reshape(NB, K))
        in_maps.append(
            {
                "keysT": keysT,
                "keysN": keysN,
                "qT": qT,
                "maskI": maskI,
                "wkT": wkT,
                "wqT": wqT,
                "wvT": wvT,
                "wkb": wkb,
                "wqb": wqb,
                "wvb": wvb,
                "eye16": eye,
                "ind2": ind2,
            }
        )
    return in_maps


def _declared_inputs(nc):
    from concourse import mybir

    names = set()
    for alloc in nc.m.functions[0].allocations:
        if isinstance(alloc, mybir.MemoryLocationSet) and alloc.kind == "ExternalInput":
            names.add(alloc.memorylocations[0].name)
    return names


def _gather(results):
    attn_out = np.concatenate(
        [np.asarray(r["attn_out"]).reshape(BS, N1, E) for r in results], axis=0
    )
    attn_w = np.concatenate(
        [np.asarray(r["attn_w"]).reshape(BS, N1, K, 1) for r in results], axis=0
    )
    return attn_out, attn_w


def kernel(**inputs):
    from concourse.bass_utils import run_bass_kernel_spmd

    nc = _CACHE.get("nc")
    if nc is None:
        nc = _build_nc()
        _CACHE["nc"] = nc

    in_maps = _make_in_maps(inputs)
    declared = _declared_inputs(nc)
    in_maps = [{k: v for k, v in m.items() if k in declared} for m in in_maps]
    trace = bool(int(os.environ.get("KERNEL_TRACE", "0")))
    try:
        res = run_bass_kernel_spmd(
            nc, in_maps, core_ids=list(range(NCORES)), trace=trace
        )
    except ModuleNotFoundError:
        # NTFF profile hook unavailable in this container; run without trace.
        res = run_bass_kernel_spmd(
            nc, in_maps, core_ids=list(range(NCORES)), trace=False
        )
    _CACHE["last_result"] = res
    return _gather(res.results)


# revision 28
# speedup vs baseline: 1.1958x; 1.0488x over previous
"""Trainium2 Bass kernel: Bahdanau (additive/tanh) attention + softmax.

Reference computation (per (b, n) pair, B=64, N1=8, K=128, E=256, H=512):
    q_proj = q @ wq^T + wq_b                  # [H]
    k_proj = keys @ wk^T + wk_b               # [K, H]
    hidden = tanh(q_proj + k_proj)            # [K, H]
    scores = hidden @ wv^T + wv_b             # [K, 1]
    scores += mask * -1e9
    attn_w = softmax(scores, axis=K)
    attn_out = sum_k attn_w * keys            # [E]

Sharding: data-parallel over batch. Each of the 8 cores gets 8 batches
= 64 (b, n) pairs; all weights replicated. kernel() shards/pre-transposes
the inputs on the host, runs one SPMD NEFF on cores 0-7 via
run_bass_kernel_spmd, and concatenates the per-core outputs.

Per-core dataflow ("H on partitions", all matmuls in float32r = full PE rate):
  * keysT [e, (bn k)] streams as the matmul moving operand in 1 MB blocks;
    wkT chunks are stationary -> k_proj^T [h_chunk, (bn k)] in PSUM.
  * q_proj^T[h, bn] (+ wk_b + wq_b) is broadcast-added into the PSUM tile by
    the DVE (stride-0 access pattern along k), then ScalarE tanh evacuates
    PSUM -> SBUF ([128, 1024] tiles).
  * scores[1, (bn k)] = wv_chunk (stationary) @ hidden, accumulated over the
    4 h-chunks in PSUM; wv_b is dropped (softmax-invariant).
  * per 16-bn tail group: scores reshaped to [16, 128] (small SBUF->SBUF
    DMA), masked, softmaxed (ScalarE exp with accum_out giving denominators),
    transposed on the PE, scattered to a block-diagonal stationary with one
    gpsimd affine_select, and contracted against keys [k, e] in 16
    accumulating matmuls -> attn_out rows. Tail work is interleaved into the
    next groups' slots via a deferred-emission scheduler so the in-order
    engine queues never stall.

The kernel streams ~12.9 MB/core from HBM, runs ~2.2 GFLOP/core of f32r
matmul, and measures ~108 us/core on hardware (loop-slope timing).
"""

import os
import sys
from contextlib import ExitStack

import numpy as np

if "/opt/trn_rl_repo" not in sys.path:
    sys.path.insert(0, "/opt/trn_rl_repo")

B, N1, K, E, H = 64, 8, 128, 256, 512
NCORES = 8
BS = B // NCORES            # batches per core
NB = BS * N1                # (b, n) pairs per core = 64
EC, ES = 2, 128             # E split into chunks of 128 (contraction tiles)
HC, HS = 4, 128             # H split into chunks of 128 (PSUM partition tiles)
BLK = 512                   # moving-operand columns per block (= 4 bn)
BPB = BLK // K              # bn per block
NBLK = NB * K // BLK        # 16 blocks

_CACHE = {}


def _build_nc(repeat=1, qmode=None):
    # q-add placement: "dve" (DVE broadcast adds), "pe" (row-packed matmuls
    # against the block indicator), or "mix" (DVE for h 0-1, PE for h 2-3 --
    # balances the two near-critical engines)
    if qmode is None:
        qmode = os.environ.get("KERNEL_QMODE", "dve")
    pe_h = {"dve": (), "pe": (0, 1, 2, 3), "mix": (2, 3)}[qmode]
    qdve = len(pe_h) < 4
    import concourse.tile as tile
    from concourse import bacc, mybir

    f32 = mybir.dt.float32
    f32r = mybir.dt.float32r
    i32 = mybir.dt.int32
    AF = mybir.ActivationFunctionType
    AX = mybir.AxisListType
    ALU = mybir.AluOpType

    nc = bacc.Bacc("TRN2", target_bir_lowering=False, debug=False)

    NTG = 4                  # tail groups of TB=16 bn
    TB = NB // NTG
    NGK = NBLK // 2          # ktile groups (block pairs)

    keysT_d = nc.dram_tensor("keysT", [EC, ES, NB * K], f32r, kind="ExternalInput")
    keysN_d = nc.dram_tensor("keysN", [K, NB, E], f32r, kind="ExternalInput")
    qT_d = nc.dram_tensor("qT", [EC, ES, NB], f32r, kind="ExternalInput")
    maskI_d = nc.dram_tensor("maskI", [NB, K], i32, kind="ExternalInput")
    wkT_d = nc.dram_tensor("wkT", [EC, ES, H], f32r, kind="ExternalInput")
    wqT_d = nc.dram_tensor("wqT", [EC, ES, H], f32r, kind="ExternalInput")
    wvT_d = nc.dram_tensor("wvT", [HS, HC], f32r, kind="ExternalInput")
    wkb_d = nc.dram_tensor("wkb", [1, H], f32, kind="ExternalInput")
    wqb_d = nc.dram_tensor("wqb", [1, H], f32, kind="ExternalInput")
    eye_d = nc.dram_tensor("eye16", [TB, TB], f32, kind="ExternalInput")
    ind_d = (nc.dram_tensor("ind2", [2 * NB, NB * K], f32r,
                            kind="ExternalInput") if pe_h else None)

    out_d = nc.dram_tensor("attn_out", [NB, E], f32, kind="ExternalOutput")
    attw_d = nc.dram_tensor("attn_w", [NB, K], f32, kind="ExternalOutput")

    def r(ap):
        return ap if ap.dtype == f32r else ap.bitcast(f32r)

    with tile.TileContext(nc) as tc, ExitStack() as ctx:
        if repeat > 1:
            # timing mode: run the whole body `repeat` times in a hardware
            # loop so device time dominates the host/axon dispatch overhead
            ctx.enter_context(
                tc.For_i(0, repeat, 1, hint_engines=(mybir.EngineType.PE,))
            )
        const = ctx.enter_context(tc.tile_pool(name="const", bufs=1))
        ktp = ctx.enter_context(tc.tile_pool(name="ktp", bufs=4))
        hidp = ctx.enter_context(tc.tile_pool(name="hidp", bufs=8))
        tailp = ctx.enter_context(tc.tile_pool(name="tailp", bufs=2))
        pk = ctx.enter_context(tc.tile_pool(name="pk", bufs=2, space="PSUM"))
        pscore = ctx.enter_context(tc.tile_pool(name="pscore", bufs=1, space="PSUM"))
        pao = ctx.enter_context(tc.tile_pool(name="pao", bufs=1, space="PSUM"))
        pmisc = ctx.enter_context(tc.tile_pool(name="pmisc", bufs=1, space="PSUM"))

        # ---------------- constants / whole-kernel inputs ----------------
        qT_sb = const.tile([ES, EC, NB], f32r)
        nc.sync.dma_start(qT_sb, qT_d.ap().rearrange("c p n -> p c n"))
        wqT_sb = const.tile([ES, EC, H], f32r)
        nc.sync.dma_start(wqT_sb, wqT_d.ap().rearrange("c p h -> p c h"))
        wkb_sb = const.tile([1, H], f32)
        nc.sync.dma_start(wkb_sb, wkb_d.ap())
        wqb_sb = const.tile([1, H], f32)
        nc.sync.dma_start(wqb_sb, wqb_d.ap())
        wkT_sb = const.tile([ES, EC, H], f32r)
        nc.sync.dma_start(wkT_sb, wkT_d.ap().rearrange("c p h -> p c h"))
        wvT_sb = const.tile([HS, HC], f32r)
        nc.sync.dma_start(wvT_sb, wvT_d.ap())
        # ind2: block-indicator stacked twice (rows 0-63 and 64-127) for the
        # row-packed pair of q-add matmuls; row 0 starts with K ones.
        # Loaded in column chunks (SWDGE, parallel to the HWDGE stream) so the
        # first q-add matmul only depends on the first chunk.
        IC = NB * K // 4
        if pe_h:
            ind_sb = const.tile([2 * NB, NB * K], f32r)
            for c in range(2):
                nc.gpsimd.dma_start(out=ind_sb[:, c * IC:(c + 1) * IC],
                                    in_=ind_d.ap()[:, c * IC:(c + 1) * IC])
        eye_sb = const.tile([TB, TB], f32)
        nc.sync.dma_start(eye_sb, eye_d.ap())
        maskf = []
        for t in range(NTG):
            mt = const.tile([TB, K], f32, tag=f"maskf{t}", name=f"maskf{t}")
            nc.gpsimd.dma_start(out=mt, in_=maskI_d.ap()[t * TB:(t + 1) * TB, :])
            maskf.append(mt)

        keysN_sb = const.tile([K, NB, E], f32r)

        if pe_h:
            ones1 = ind_sb[0:1, 0:NB]
            bsum = const.tile([1, H], f32r)
            nc.vector.tensor_add(bsum, wkb_sb, wqb_sb)

        # ---------------- q projection ----------------
        if True:
            # qpT[h, bn] (+ biases) for DVE broadcast-adds into PSUM
            bsumT = const.tile([HS, HC], f32)
            nc.sync.dma_start(
                bsumT, wkb_d.ap().rearrange("o (c p) -> p (o c)", p=HS))
            wqbT = const.tile([HS, HC], f32)
            nc.sync.dma_start(
                wqbT, wqb_d.ap().rearrange("o (c p) -> p (o c)", p=HS))
            nc.vector.tensor_add(bsumT, bsumT, wqbT)
            qpT_ps = pmisc.tile([HS, HC, NB], f32, tag="pm")
            for hh in range(HC):
                for e in range(EC):
                    nc.tensor.matmul(
                        qpT_ps[:, hh, :],
                        r(wqT_sb[:, e, hh * HS:(hh + 1) * HS]),
                        r(qT_sb[:, e, :]),
                        start=(e == 0), stop=(e == EC - 1))
            qpT_sb = const.tile([HS, HC, NB], f32)
            for hh in range(HC):
                nc.vector.tensor_scalar_add(qpT_sb[:, hh, :], qpT_ps[:, hh, :],
                                            bsumT[:, hh:hh + 1])
        if pe_h:
            qp_ps = pmisc.tile([NB, H], f32, tag="pm")
            nc.tensor.matmul(qp_ps, r(qT_sb[:, 0, :]), r(wqT_sb[:, 0, :]),
                             start=True, stop=False)
            nc.tensor.matmul(qp_ps, r(qT_sb[:, 1, :]), r(wqT_sb[:, 1, :]),
                             start=False, stop=False)
            nc.tensor.matmul(qp_ps, r(ones1), r(bsum), start=False, stop=True)
            qp2_sb = const.tile([2 * NB, H], f32r)
            nc.vector.tensor_copy(qp2_sb[0:NB, :], qp_ps)
            nc.vector.tensor_copy(qp2_sb[NB:2 * NB, :], qp_ps)

        scores_row = const.tile([1, NBLK * BLK], f32)

        # Deferred-emission scheduler: closures registered at (group, h) flush
        # points so the in-order PE queue never waits on ACT/DVE results of
        # the tile it just produced, and the softmax/attn_out tail of each
        # 16-bn group is spread across the next groups' slots.
        sched = {}

        def at(gk, h, fn):
            sched.setdefault((gk, h), []).append(fn)

        def flush(gk, h):
            for fn in sched.pop((gk, h), []):
                fn()

        state = {}

        def make_sc_mm(gk, half, h, hd):
            def f():
                key = f"sc{gk}"
                if key not in state:
                    state[key] = pscore.tile([1, 2 * BLK], f32, tag="sc",
                                             name=f"sc_ps_{gk}")
                sc = state[key][:, half * BLK:(half + 1) * BLK]
                nc.tensor.matmul(sc, r(wvT_sb[:, h:h + 1]), r(hd),
                                 start=(h == 0), stop=(h == HC - 1),
                                 skip_group_check=True)
            return f

        def make_sc_evac(gk):
            def f():
                sc = state.pop(f"sc{gk}")
                nc.scalar.copy(
                    scores_row[:, 2 * gk * BLK:(2 * gk + 2) * BLK], sc)
            return f

        def make_tail_a(t):
            def f():
                sb = tailp.tile([TB, K], f32, tag="scbk", name=f"scbk{t}")
                nc.sync.dma_start(
                    sb,
                    scores_row[0:1, t * TB * K:(t + 1) * TB * K].rearrange(
                        "p (n k) -> p n k", k=K))
                ms = tailp.tile([TB, K], f32, tag="maskS", name=f"maskS{t}")
                nc.vector.tensor_scalar_mul(ms, maskf[t], -1.0e9)
                nc.vector.tensor_add(sb, sb, ms)
                nm = tailp.tile([TB, 1], f32, tag="negmx", name=f"negmx{t}")
                nc.vector.tensor_reduce(nm, sb, axis=AX.X, op=ALU.max,
                                        negate=True)
                state[f"sb{t}"] = sb
                state[f"nm{t}"] = nm
            return f

        def make_tail_b(t):
            def f():
                sb = state.pop(f"sb{t}")
                nm = state.pop(f"nm{t}")
                aw = tailp.tile([TB, K], f32, tag="aw", name=f"aw{t}")
                se = tailp.tile([TB, 1], f32, tag="se", name=f"se{t}")
                nc.scalar.activation(aw, sb, AF.Exp, bias=nm, accum_out=se)
                rs = tailp.tile([TB, 1], f32, tag="rs", name=f"rs{t}")
                nc.vector.reciprocal(rs, se)
                nc.vector.tensor_scalar_mul(aw, aw, rs)
                nc.sync.dma_start(attw_d.ap()[t * TB:(t + 1) * TB, :], aw)
                state[f"aw{t}"] = aw
            return f

        def make_tail_c(t):
            def f():
                aw = state.pop(f"aw{t}")
                tp = pmisc.tile([K, TB], f32, tag="pm", name=f"tp{t}")
                nc.tensor.transpose(tp, aw, eye_sb)
                at_sb = tailp.tile([K, TB], f32r, tag="at", name=f"at{t}")
                nc.vector.tensor_copy(at_sb, tp)
                ad = tailp.tile([K, TB, TB], f32r, tag="ad", name=f"ad{t}")
                nc.gpsimd.affine_select(
                    out=ad, in_=at_sb.unsqueeze(2).broadcast_to([K, TB, TB]),
                    pattern=[[1, TB], [-1, TB]], base=0, channel_multiplier=0,
                    compare_op=ALU.is_equal, fill=0.0)
                state[f"ad{t}"] = ad
            return f

        def make_tail_d(t):
            def f():
                ad = state.pop(f"ad{t}")
                ao = pao.tile([TB, E], f32, tag="ao", name=f"ao{t}")
                for i in range(TB):
                    nc.tensor.matmul(ao, r(ad[:, i, :]),
                                     r(keysN_sb[:, t * TB + i, :]),
                                     start=(i == 0), stop=(i == TB - 1))
                osb = tailp.tile([TB, E], f32, tag="osb", name=f"osb{t}")
                nc.vector.tensor_copy(osb, ao)
                nc.sync.dma_start(out_d.ap()[t * TB:(t + 1) * TB, :], osb)
            return f

        # ---------------- main loop over ktile-group (pair of blocks) ------
        for gk in range(NGK):
            j0, j1 = 2 * gk, 2 * gk + 1
            ktile = ktp.tile([ES, EC, 2 * BLK], f32r, tag="ktile")
            nc.sync.dma_start(
                ktile,
                keysT_d.ap()[:, :, j0 * BLK:(j1 + 1) * BLK].rearrange(
                    "c p n -> p c n"))
            if gk in (2, 4) and pe_h:
                c = gk // 2 + 1
                nc.gpsimd.dma_start(out=ind_sb[:, c * IC:(c + 1) * IC],
                                    in_=ind_d.ap()[:, c * IC:(c + 1) * IC])
            if gk % 2 == 0:
                t = gk // 2
                nc.sync.dma_start(
                    keysN_sb[:, t * TB:(t + 1) * TB, :],
                    keysN_d.ap()[:, t * TB:(t + 1) * TB, :])
            for h in range(HC):
                hsl = slice(h * HS, (h + 1) * HS)
                kp = pk.tile([ES, 2 * BLK], f32, tag="kp", name=f"kp_{gk}_{h}")
                kp0, kp1 = kp[:, 0:BLK], kp[:, BLK:2 * BLK]
                nc.tensor.matmul(kp0, r(wkT_sb[:, 0, hsl]),
                                 r(ktile[:, 0, 0:BLK]), start=True, stop=False)
                nc.tensor.matmul(kp1, r(wkT_sb[:, 0, hsl]),
                                 r(ktile[:, 0, BLK:2 * BLK]),
                                 start=True, stop=False)
                dve_h = h not in pe_h
                nc.tensor.matmul(kp0, r(wkT_sb[:, 1, hsl]),
                                 r(ktile[:, 1, 0:BLK]), start=False, stop=dve_h)
                nc.tensor.matmul(kp1, r(wkT_sb[:, 1, hsl]),
                                 r(ktile[:, 1, BLK:2 * BLK]),
                                 start=False, stop=dve_h)
                if dve_h:
                    qb = qpT_sb[:, h, 8 * gk:8 * gk + 8].unsqueeze(
                        2).broadcast_to([HS, 8, K])
                    nc.vector.tensor_add(
                        kp.rearrange("p (n k) -> p n k", k=K), kp.rearrange(
                            "p (n k) -> p n k", k=K), qb)
                else:
                    # row-packed pair: partitions 0-63/64-127 run concurrently
                    nc.tensor.matmul(kp0, r(qp2_sb[0:NB, hsl]),
                                     r(ind_sb[0:NB, j0 * BLK:(j0 + 1) * BLK]),
                                     start=False, stop=True)
                    nc.tensor.matmul(kp1, r(qp2_sb[NB:2 * NB, hsl]),
                                     r(ind_sb[NB:2 * NB, j1 * BLK:(j1 + 1) * BLK]),
                                     start=False, stop=True)
                hid = hidp.tile([ES, 2 * BLK], f32r, tag="hid")
                nc.scalar.activation(hid, kp, AF.Tanh)
                flush(gk, h)
                nxt = (gk, h + 1) if h + 1 < HC else (gk + 1, 0)
                at(*nxt, make_sc_mm(gk, 0, h, hid[:, 0:BLK]))
                at(*nxt, make_sc_mm(gk, 1, h, hid[:, BLK:2 * BLK]))
            at(gk + 1, 0, make_sc_evac(gk))
            if gk % 2 == 1:
                t = gk // 2
                at(gk + 1, 1, make_tail_a(t))
                at(gk + 1, 2, make_tail_b(t))
                at(gk + 1, 3, make_tail_c(t))
                at(gk + 2, 0, make_tail_d(t))

        # post-loop: flush everything remaining in registration order
        for key in sorted(sched.keys()):
            for fn in list(sched.pop(key)):
                fn()

    nc.compile()
    return nc


def _make_in_maps(inputs):
    queries = np.asarray(inputs["queries"], np.float32)
    keys = np.asarray(inputs["keys"], np.float32)
    masks = np.asarray(inputs["cum_subs_masks"], np.int32)
    wk_w = np.asarray(inputs["wk_w"], np.float32)
    wk_b = np.asarray(inputs["wk_b"], np.float32)
    wq_w = np.asarray(inputs["wq_w"], np.float32)
    wq_b = np.asarray(inputs["wq_b"], np.float32)
    wv_w = np.asarray(inputs["wv_w"], np.float32)
    wv_b = np.asarray(inputs["wv_b"], np.float32)

    wkT = np.ascontiguousarray(wk_w.T).reshape(EC, ES, H)
    wqT = np.ascontiguousarray(wq_w.T).reshape(EC, ES, H)
    wvT = np.ascontiguousarray(wv_w.reshape(HC, HS).T)
    wkb = np.ascontiguousarray(wk_b.reshape(1, H))
    wqb = np.ascontiguousarray(wq_b.reshape(1, H))
    wvb = np.ascontiguousarray(wv_b.reshape(1, 1))
    eye = np.eye(16, dtype=np.float32)
    ind = np.kron(np.eye(NB, dtype=np.float32), np.ones((1, K), np.float32))
    ind2 = np.ascontiguousarray(np.vstack([ind, ind]))

    in_maps = []
    for c in range(NCORES):
        sl = slice(c * BS, (c + 1) * BS)
        kc = keys[sl].reshape(NB, K, E)
        keysT = np.ascontiguousarray(kc.transpose(2, 0, 1)).reshape(EC, ES, NB * K)
        keysN = np.ascontiguousarray(kc.transpose(1, 0, 2))
        qT = np.ascontiguousarray(queries[sl].reshape(NB, E).T).reshape(EC, ES, NB)
        maskI = np.ascontiguousarray(masks[sl].reshape(NB, K))
        in_maps.append(
            {
                "keysT": keysT,
                "keysN": keysN,
                "qT": qT,
                "maskI": maskI,
                "wkT": wkT,
                "wqT": wqT,
                "wvT": wvT,
                "wkb": wkb,
                "wqb": wqb,
                "wvb": wvb,
                "eye16": eye,
                "ind2": ind2,
            }
        )
    return in_maps


def _declared_inputs(nc):
    from concourse import mybir

    names = set()
    for alloc in nc.m.functions[0].allocations:
        if isinstance(alloc, mybir.MemoryLocationSet) and alloc.kind == "ExternalInput":
            names.add(alloc.memorylocations[0].name)
    return names


def _gather(results):
    attn_out = np.concatenate(
        [np.asarray(r["attn_out"]).reshape(BS, N1, E) for r in results], axis=0
    )
    attn_w = np.concatenate(
        [np.asarray(r["attn_w"]).reshape(BS, N1, K, 1) for r in results], axis=0
    )
    return attn_out, attn_w


def kernel(**inputs):
    from concourse.bass_utils import run_bass_kernel_spmd

    nc = _CACHE.get("nc")
    if nc is None:
        nc = _build_nc()
        _CACHE["nc"] = nc

    in_maps = _make_in_maps(inputs)
    declared = _declared_inputs(nc)
    in_maps = [{k: v for k, v in m.items() if k in declared} for m in in_maps]
    trace = bool(int(os.environ.get("KERNEL_TRACE", "0")))
    try:
        res = run_bass_kernel_spmd(
            nc, in_maps, core_ids=list(range(NCORES)), trace=trace
        )
    except ModuleNotFoundError:
        # NTFF profile hook unavailable in this container; run without trace.
        res = run_bass_kernel_spmd(
            nc, in_maps, core_ids=list(range(NCORES)), trace=False
        )
    _CACHE["last_result"] = res
    return _gather(res.results)


# revision 29
# speedup vs baseline: 1.2644x; 1.0574x over previous
"""Trainium2 Bass kernel: Bahdanau (additive/tanh) attention + softmax.

Reference computation (per (b, n) pair, B=64, N1=8, K=128, E=256, H=512):
    q_proj = q @ wq^T + wq_b                  # [H]
    k_proj = keys @ wk^T + wk_b               # [K, H]
    hidden = tanh(q_proj + k_proj)            # [K, H]
    scores = hidden @ wv^T + wv_b             # [K, 1]
    scores += mask * -1e9
    attn_w = softmax(scores, axis=K)
    attn_out = sum_k attn_w * keys            # [E]

Sharding: data-parallel over batch. Each of the 8 cores gets 8 batches
= 64 (b, n) pairs; all weights replicated. kernel() shards/pre-transposes
the inputs on the host, runs one SPMD NEFF on cores 0-7 via
run_bass_kernel_spmd, and concatenates the per-core outputs.

Per-core dataflow ("H on partitions", all matmuls in float32r = full PE rate):
  * keysT [e, (bn k)] streams as the matmul moving operand in 1 MB blocks;
    wkT chunks are stationary -> k_proj^T [h_chunk, (bn k)] in PSUM.
  * q_proj^T[h, bn] (+ wk_b + wq_b) is broadcast-added into the PSUM tile by
    the DVE (stride-0 access pattern along k), then ScalarE tanh evacuates
    PSUM -> SBUF ([128, 1024] tiles).
  * scores[1, (bn k)] = wv_chunk (stationary) @ hidden, accumulated over the
    4 h-chunks in PSUM; wv_b is dropped (softmax-invariant).
  * per 16-bn tail group: scores reshaped to [16, 128] (small SBUF->SBUF
    DMA), masked, softmaxed (ScalarE exp with accum_out giving denominators),
    transposed on the PE, scattered to a block-diagonal stationary with one
    gpsimd affine_select, and contracted against keys [k, e] in 16
    accumulating matmuls -> attn_out rows. Tail work is interleaved into the
    next groups' slots via a deferred-emission scheduler so the in-order
    engine queues never stall.

The kernel streams ~12.9 MB/core from HBM, runs ~2.2 GFLOP/core of f32r
matmul, and measures ~108 us/core on hardware (loop-slope timing).
"""

import os
import sys
from contextlib import ExitStack

import numpy as np

if "/opt/trn_rl_repo" not in sys.path:
    sys.path.insert(0, "/opt/trn_rl_repo")

B, N1, K, E, H = 64, 8, 128, 256, 512
NCORES = 8
BS = B // NCORES            # batches per core
NB = BS * N1                # (b, n) pairs per core = 64
EC, ES = 2, 128             # E split into chunks of 128 (contraction tiles)
HC, HS = 4, 128             # H split into chunks of 128 (PSUM partition tiles)
BLK = 512                   # moving-operand columns per block (= 4 bn)
BPB = BLK // K              # bn per block
NBLK = NB * K // BLK        # 16 blocks

_CACHE = {}


def _build_nc(repeat=1, qmode=None):
    # q-add placement: "dve" (DVE broadcast adds), "pe" (row-packed matmuls
    # against the block indicator), or "mix" (DVE for h 0-1, PE for h 2-3 --
    # balances the two near-critical engines)
    if qmode is None:
        qmode = os.environ.get("KERNEL_QMODE", "dve")
    pe_h = {"dve": (), "pe": (0, 1, 2, 3), "mix": (2, 3)}[qmode]
    qdve = len(pe_h) < 4
    import concourse.tile as tile
    from concourse import bacc, mybir

    f32 = mybir.dt.float32
    f32r = mybir.dt.float32r
    i32 = mybir.dt.int32
    AF = mybir.ActivationFunctionType
    AX = mybir.AxisListType
    ALU = mybir.AluOpType

    nc = bacc.Bacc("TRN2", target_bir_lowering=False, debug=False)

    NTG = 4                  # tail groups of TB=16 bn
    TB = NB // NTG
    NGK = NBLK // 2          # ktile groups (block pairs)

    keysT_d = nc.dram_tensor("keysT", [EC, ES, NB * K], f32r, kind="ExternalInput")
    keysN_d = nc.dram_tensor("keysN", [K, NB, E], f32r, kind="ExternalInput")
    qT_d = nc.dram_tensor("qT", [EC, ES, NB], f32r, kind="ExternalInput")
    maskI_d = nc.dram_tensor("maskI", [NB, K], i32, kind="ExternalInput")
    wkT_d = nc.dram_tensor("wkT", [EC, ES, H], f32r, kind="ExternalInput")
    wqT_d = nc.dram_tensor("wqT", [EC, ES, H], f32r, kind="ExternalInput")
    wvT_d = nc.dram_tensor("wvT", [HS, HC], f32r, kind="ExternalInput")
    wkb_d = nc.dram_tensor("wkb", [1, H], f32, kind="ExternalInput")
    wqb_d = nc.dram_tensor("wqb", [1, H], f32, kind="ExternalInput")
    eye_d = nc.dram_tensor("eye16", [TB, TB], f32, kind="ExternalInput")
    ind_d = (nc.dram_tensor("ind2", [2 * NB, NB * K], f32r,
                            kind="ExternalInput") if pe_h else None)

    out_d = nc.dram_tensor("attn_out", [NB, E], f32, kind="ExternalOutput")
    attw_d = nc.dram_tensor("attn_w", [NB, K], f32, kind="ExternalOutput")

    def r(ap):
        return ap if ap.dtype == f32r else ap.bitcast(f32r)

    with tile.TileContext(nc) as tc, ExitStack() as ctx:
        if repeat > 1:
            # timing mode: run the whole body `repeat` times in a hardware
            # loop so device time dominates the host/axon dispatch overhead
            ctx.enter_context(
                tc.For_i(0, repeat, 1, hint_engines=(mybir.EngineType.PE,))
            )
        const = ctx.enter_context(tc.tile_pool(name="const", bufs=1))
        ktp = ctx.enter_context(tc.tile_pool(name="ktp", bufs=4))
        hidp = ctx.enter_context(tc.tile_pool(name="hidp", bufs=8))
        tailp = ctx.enter_context(tc.tile_pool(name="tailp", bufs=2))
        pk = ctx.enter_context(tc.tile_pool(name="pk", bufs=2, space="PSUM"))
        pscore = ctx.enter_context(tc.tile_pool(name="pscore", bufs=1, space="PSUM"))
        pao = ctx.enter_context(tc.tile_pool(name="pao", bufs=1, space="PSUM"))
        pmisc = ctx.enter_context(tc.tile_pool(name="pmisc", bufs=1, space="PSUM"))

        # ---------------- constants / whole-kernel inputs ----------------
        qT_sb = const.tile([ES, EC, NB], f32r)
        nc.sync.dma_start(qT_sb, qT_d.ap().rearrange("c p n -> p c n"))
        wqT_sb = const.tile([ES, EC, H], f32r)
        nc.sync.dma_start(wqT_sb, wqT_d.ap().rearrange("c p h -> p c h"))
        wkb_sb = const.tile([1, H], f32)
        nc.sync.dma_start(wkb_sb, wkb_d.ap())
        wqb_sb = const.tile([1, H], f32)
        nc.sync.dma_start(wqb_sb, wqb_d.ap())
        wkT_sb = const.tile([ES, EC, H], f32r)
        nc.sync.dma_start(wkT_sb, wkT_d.ap().rearrange("c p h -> p c h"))
        wvT_sb = const.tile([HS, HC], f32r)
        nc.sync.dma_start(wvT_sb, wvT_d.ap())
        # ind2: block-indicator stacked twice (rows 0-63 and 64-127) for the
        # row-packed pair of q-add matmuls; row 0 starts with K ones.
        # Loaded in column chunks (SWDGE, parallel to the HWDGE stream) so the
        # first q-add matmul only depends on the first chunk.
        IC = NB * K // 4
        if pe_h:
            ind_sb = const.tile([2 * NB, NB * K], f32r)
            for c in range(2):
                nc.gpsimd.dma_start(out=ind_sb[:, c * IC:(c + 1) * IC],
                                    in_=ind_d.ap()[:, c * IC:(c + 1) * IC])
        eye_sb = const.tile([TB, TB], f32)
        nc.sync.dma_start(eye_sb, eye_d.ap())
        maskf = []
        for t in range(NTG):
            mt = const.tile([TB, K], f32, tag=f"maskf{t}", name=f"maskf{t}")
            nc.gpsimd.dma_start(out=mt, in_=maskI_d.ap()[t * TB:(t + 1) * TB, :])
            maskf.append(mt)

        keysN_sb = const.tile([K, NB, E], f32r)

        if pe_h:
            ones1 = ind_sb[0:1, 0:NB]
            bsum = const.tile([1, H], f32r)
            nc.vector.tensor_add(bsum, wkb_sb, wqb_sb)

        # ---------------- q projection ----------------
        if True:
            # qpT[h, bn] (+ biases) for DVE broadcast-adds into PSUM
            bsumT = const.tile([HS, HC], f32)
            nc.sync.dma_start(
                bsumT, wkb_d.ap().rearrange("o (c p) -> p (o c)", p=HS))
            wqbT = const.tile([HS, HC], f32)
            nc.sync.dma_start(
                wqbT, wqb_d.ap().rearrange("o (c p) -> p (o c)", p=HS))
            nc.vector.tensor_add(bsumT, bsumT, wqbT)
            qpT_ps = pmisc.tile([HS, HC, NB], f32, tag="pm")
            for hh in range(HC):
                for e in range(EC):
                    nc.tensor.matmul(
                        qpT_ps[:, hh, :],
                        r(wqT_sb[:, e, hh * HS:(hh + 1) * HS]),
                        r(qT_sb[:, e, :]),
                        start=(e == 0), stop=(e == EC - 1))
            qpT_sb = const.tile([HS, HC, NB], f32)
            for hh in range(HC):
                nc.vector.tensor_scalar_add(qpT_sb[:, hh, :], qpT_ps[:, hh, :],
                                            bsumT[:, hh:hh + 1])
        if pe_h:
            qp_ps = pmisc.tile([NB, H], f32, tag="pm")
            nc.tensor.matmul(qp_ps, r(qT_sb[:, 0, :]), r(wqT_sb[:, 0, :]),
                             start=True, stop=False)
            nc.tensor.matmul(qp_ps, r(qT_sb[:, 1, :]), r(wqT_sb[:, 1, :]),
                             start=False, stop=False)
            nc.tensor.matmul(qp_ps, r(ones1), r(bsum), start=False, stop=True)
            qp2_sb = const.tile([2 * NB, H], f32r)
            nc.vector.tensor_copy(qp2_sb[0:NB, :], qp_ps)
            nc.vector.tensor_copy(qp2_sb[NB:2 * NB, :], qp_ps)

        scores_row = const.tile([1, NBLK * BLK], f32)

        # Deferred-emission scheduler: closures registered at (group, h) flush
        # points so the in-order PE queue never waits on ACT/DVE results of
        # the tile it just produced, and the softmax/attn_out tail of each
        # 16-bn group is spread across the next groups' slots.
        sched = {}

        def at(gk, h, fn):
            sched.setdefault((gk, h), []).append(fn)

        def flush(gk, h):
            for fn in sched.pop((gk, h), []):
                fn()

        state = {}

        def make_sc_mm(gk, half, h, hd):
            def f():
                key = f"sc{gk}"
                if key not in state:
                    state[key] = pscore.tile([1, 2 * BLK], f32, tag="sc",
                                             name=f"sc_ps_{gk}")
                sc = state[key][:, half * BLK:(half + 1) * BLK]
                nc.tensor.matmul(sc, r(wvT_sb[:, h:h + 1]), r(hd),
                                 start=(h == 0), stop=(h == HC - 1),
                                 skip_group_check=True)
            return f

        def make_sc_evac(gk):
            def f():
                sc = state.pop(f"sc{gk}")
                nc.scalar.copy(
                    scores_row[:, 2 * gk * BLK:(2 * gk + 2) * BLK], sc)
            return f

        def make_tail_a(t):
            def f():
                sb = tailp.tile([TB, K], f32, tag="scbk", name=f"scbk{t}")
                nc.sync.dma_start(
                    sb,
                    scores_row[0:1, t * TB * K:(t + 1) * TB * K].rearrange(
                        "p (n k) -> p n k", k=K))
                ms = tailp.tile([TB, K], f32, tag="maskS", name=f"maskS{t}")
                nc.vector.tensor_scalar_mul(ms, maskf[t], -1.0e9)
                nc.vector.tensor_add(sb, sb, ms)
                nm = tailp.tile([TB, 1], f32, tag="negmx", name=f"negmx{t}")
                nc.vector.tensor_reduce(nm, sb, axis=AX.X, op=ALU.max,
                                        negate=True)
                state[f"sb{t}"] = sb
                state[f"nm{t}"] = nm
            return f

        def make_tail_b(t):
            def f():
                sb = state.pop(f"sb{t}")
                nm = state.pop(f"nm{t}")
                aw = tailp.tile([TB, K], f32, tag="aw", name=f"aw{t}")
                se = tailp.tile([TB, 1], f32, tag="se", name=f"se{t}")
                nc.scalar.activation(aw, sb, AF.Exp, bias=nm, accum_out=se)
                rs = tailp.tile([TB, 1], f32, tag="rs", name=f"rs{t}")
                nc.vector.reciprocal(rs, se)
                nc.vector.tensor_scalar_mul(aw, aw, rs)
                nc.sync.dma_start(attw_d.ap()[t * TB:(t + 1) * TB, :], aw)
                state[f"aw{t}"] = aw
            return f

        def make_tail_c(t):
            def f():
                aw = state.pop(f"aw{t}")
                tp = pmisc.tile([K, TB], f32, tag="pm", name=f"tp{t}")
                nc.tensor.transpose(tp, aw, eye_sb)
                at_sb = tailp.tile([K, TB], f32r, tag="at", name=f"at{t}")
                nc.vector.tensor_copy(at_sb, tp)
                ad = tailp.tile([K, TB, TB], f32r, tag="ad", name=f"ad{t}")
                nc.gpsimd.affine_select(
                    out=ad, in_=at_sb.unsqueeze(2).broadcast_to([K, TB, TB]),
                    pattern=[[1, TB], [-1, TB]], base=0, channel_multiplier=0,
                    compare_op=ALU.is_equal, fill=0.0)
                state[f"ad{t}"] = ad
            return f

        def make_tail_d(t):
            def f():
                ad = state.pop(f"ad{t}")
                ao = pao.tile([TB, E], f32, tag="ao", name=f"ao{t}")
                for i in range(TB):
                    nc.tensor.matmul(ao, r(ad[:, i, :]),
                                     r(keysN_sb[:, t * TB + i, :]),
                                     start=(i == 0), stop=(i == TB - 1))
                osb = tailp.tile([TB, E], f32, tag="osb", name=f"osb{t}")
                nc.vector.tensor_copy(osb, ao)
                nc.sync.dma_start(out_d.ap()[t * TB:(t + 1) * TB, :], osb)
            return f

        # ---------------- main loop over ktile-group (pair of blocks) ------
        for gk in range(NGK):
            j0, j1 = 2 * gk, 2 * gk + 1
            ktile = ktp.tile([ES, EC, 2 * BLK], f32r, tag="ktile")
            nc.sync.dma_start(
                ktile,
                keysT_d.ap()[:, :, j0 * BLK:(j1 + 1) * BLK].rearrange(
                    "c p n -> p c n"))
            if gk in (2, 4) and pe_h:
                c = gk // 2 + 1
                nc.gpsimd.dma_start(out=ind_sb[:, c * IC:(c + 1) * IC],
                                    in_=ind_d.ap()[:, c * IC:(c + 1) * IC])
            if gk % 2 == 0:
                t = gk // 2
                nc.sync.dma_start(
                    keysN_sb[:, t * TB:(t + 1) * TB, :],
                    keysN_d.ap()[:, t * TB:(t + 1) * TB, :])
            for h in range(HC):
                hsl = slice(h * HS, (h + 1) * HS)
                kp = pk.tile([ES, 2 * BLK], f32, tag="kp", name=f"kp_{gk}_{h}")
                kp0, kp1 = kp[:, 0:BLK], kp[:, BLK:2 * BLK]
                nc.tensor.matmul(kp0, r(wkT_sb[:, 0, hsl]),
                                 r(ktile[:, 0, 0:BLK]), start=True, stop=False)
                nc.tensor.matmul(kp1, r(wkT_sb[:, 0, hsl]),
                                 r(ktile[:, 0, BLK:2 * BLK]),
                                 start=True, stop=False)
                dve_h = h not in pe_h
                nc.tensor.matmul(kp0, r(wkT_sb[:, 1, hsl]),
                                 r(ktile[:, 1, 0:BLK]), start=False, stop=dve_h)
                nc.tensor.matmul(kp1, r(wkT_sb[:, 1, hsl]),
                                 r(ktile[:, 1, BLK:2 * BLK]),
                                 start=False, stop=dve_h)
                if dve_h:
                    qb = qpT_sb[:, h, 8 * gk:8 * gk + 8].unsqueeze(
                        2).broadcast_to([HS, 8, K])
                    nc.vector.tensor_add(
                        kp.rearrange("p (n k) -> p n k", k=K), kp.rearrange(
                            "p (n k) -> p n k", k=K), qb)
                else:
                    # row-packed pair: partitions 0-63/64-127 run concurrently
                    nc.tensor.matmul(kp0, r(qp2_sb[0:NB, hsl]),
                                     r(ind_sb[0:NB, j0 * BLK:(j0 + 1) * BLK]),
                                     start=False, stop=True)
                    nc.tensor.matmul(kp1, r(qp2_sb[NB:2 * NB, hsl]),
                                     r(ind_sb[NB:2 * NB, j1 * BLK:(j1 + 1) * BLK]),
                                     start=False, stop=True)
                hid = hidp.tile([ES, 2 * BLK], f32r, tag="hid")
                nc.scalar.activation(hid, kp, AF.Tanh)
                flush(gk, h)
                # two-slot deferral: gives tanh ~2 PE slots of headroom
                nxt = divmod(4 * gk + h + 2, HC)
                at(*nxt, make_sc_mm(gk, 0, h, hid[:, 0:BLK]))
                at(*nxt, make_sc_mm(gk, 1, h, hid[:, BLK:2 * BLK]))
            at(gk + 1, 1, make_sc_evac(gk))
            if gk % 2 == 1:
                t = gk // 2
                at(gk + 1, 2, make_tail_a(t))
                at(gk + 1, 3, make_tail_b(t))
                at(gk + 2, 0, make_tail_c(t))
                at(gk + 2, 1, make_tail_d(t))

        # post-loop: flush everything remaining in registration order
        for key in sorted(sched.keys()):
            for fn in list(sched.pop(key)):
                fn()

    nc.compile()
    return nc


def _make_in_maps(inputs):
    queries = np.asarray(inputs["queries"], np.float32)
    keys = np.asarray(inputs["keys"], np.float32)
    masks = np.asarray(inputs["cum_subs_masks"], np.int32)
    wk_w = np.asarray(inputs["wk_w"], np.float32)
    wk_b = np.asarray(inputs["wk_b"], np.float32)
    wq_w = np.asarray(inputs["wq_w"], np.float32)
    wq_b = np.asarray(inputs["wq_b"], np.float32)
    wv_w = np.asarray(inputs["wv_w"], np.float32)
    wv_b = np.asarray(inputs["wv_b"], np.float32)

    wkT = np.ascontiguousarray(wk_w.T).reshape(EC, ES, H)
    wqT = np.ascontiguousarray(wq_w.T).reshape(EC, ES, H)
    wvT = np.ascontiguousarray(wv_w.reshape(HC, HS).T)
    wkb = np.ascontiguousarray(wk_b.reshape(1, H))
    wqb = np.ascontiguousarray(wq_b.reshape(1, H))
    wvb = np.ascontiguousarray(wv_b.reshape(1, 1))
    eye = np.eye(16, dtype=np.float32)
    ind = np.kron(np.eye(NB, dtype=np.float32), np.ones((1, K), np.float32))
    ind2 = np.ascontiguousarray(np.vstack([ind, ind]))

    in_maps = []
    for c in range(NCORES):
        sl = slice(c * BS, (c + 1) * BS)
        kc = keys[sl].reshape(NB, K, E)
        keysT = np.ascontiguousarray(kc.transpose(2, 0, 1)).reshape(EC, ES, NB * K)
        keysN = np.ascontiguousarray(kc.transpose(1, 0, 2))
        qT = np.ascontiguousarray(queries[sl].reshape(NB, E).T).reshape(EC, ES, NB)
        maskI = np.ascontiguousarray(masks[sl].reshape(NB, K))
        in_maps.append(
            {
                "keysT": keysT,
                "keysN": keysN,
                "qT": qT,
                "maskI": maskI,
                "wkT": wkT,
                "wqT": wqT,
                "wvT": wvT,
                "wkb": wkb,
                "wqb": wqb,
                "wvb": wvb,
                "eye16": eye,
                "ind2": ind2,
            }
        )
    return in_maps


def _declared_inputs(nc):
    from concourse import mybir

    names = set()
    for alloc in nc.m.functions[0].allocations:
        if isinstance(alloc, mybir.MemoryLocationSet) and alloc.kind == "ExternalInput":
            names.add(alloc.memorylocations[0].name)
    return names


def _gather(results):
    attn_out = np.concatenate(
        [np.asarray(r["attn_out"]).reshape(BS, N1, E) for r in results], axis=0
    )
    attn_w = np.concatenate(
        [np.asarray(r["attn_w"]).reshape(BS, N1, K, 1) for r in results], axis=0
    )
    return attn_out, attn_w


def kernel(**inputs):
    from concourse.bass_utils import run_bass_kernel_spmd

    nc = _CACHE.get("nc")
    if nc is None:
        nc = _build_nc()
        _CACHE["nc"] = nc

    in_maps = _make_in_maps(inputs)
    declared = _declared_inputs(nc)
    in_maps = [{k: v for k, v in m.items() if k in declared} for m in in_maps]
    trace = bool(int(os.environ.get("KERNEL_TRACE", "0")))
    try:
        res = run_bass_kernel_spmd(
            nc, in_maps, core_ids=list(range(NCORES)), trace=trace
        )
    except ModuleNotFoundError:
        # NTFF profile hook unavailable in this container; run without trace.
        res = run_bass_kernel_spmd(
            nc, in_maps, core_ids=list(range(NCORES)), trace=False
        )
    _CACHE["last_result"] = res
    return _gather(res.results)
